# revision 29
# baseline (speedup 1.0000x reference)
"""Trainium2 Bass kernel for nn_Distiller attention-distillation loss.

Computes, for f_s, f_t of shape [8, 256, 32, 32]:
    q = k_tokens(f_s), k = tokens(f_t), v = tokens(f_s)   (8 heads, d=32, n=1024)
    out = softmax(q @ k^T) @ v          (per batch, per head; unscaled logits)
    loss = mean((out_img - f_t)^2)      (scalar)

Sharding: data-parallel over batch b — one batch element per NeuronCore (8
cores).  Each core computes its partial sum of squared errors; the host sums
the 8 partials and divides by the element count.  The mean is layout
invariant, so the loss is computed in token space and the final
'b h (x y) d -> b (h d) x y' rearrange is never materialized.

Per-core algorithm (all in [d, n]-major "transposed token" layouts so that
no input transposes are needed):
  simT[j, i] = sum_d kT[d, j] * qT[d, i]        (PE, bf16 inputs, fp32 psum)
  expT = exp(simT)                               (ACT, psum -> sbuf bf16)
  [u; s][d_aug, i] = [v_tok | 1s]^T-style matmul: stationary = v_tok[j, 33]
       (v tokens with an appended ones-column), moving = expT chunks,
       accumulated over j in psum.  Row 32 is the softmax denominator s[i].
  loss_part += sum((u/s - tT)^2)                 (DVE + custom ops)
"""

import numpy as np

import concourse.bass as bass
import concourse.bacc as bacc
import concourse.tile as tile
import concourse.mybir as mybir
from concourse.bass_utils import run_bass_kernel_spmd

F32 = mybir.dt.float32
BF16 = mybir.dt.bfloat16
AF = mybir.ActivationFunctionType
ALU = mybir.AluOpType

B = 8          # batch (== number of cores)
H = 8          # heads
D = 32         # head dim
N = 1024       # tokens (32*32)
C = H * D      # channels = 256
NCORES = 8
TOTAL_ELEMS = B * C * 32 * 32  # 2097152


def _body(ctx, tc, fs, ft, out_dram):
    nc = tc.nc

    inp = ctx.enter_context(tc.tile_pool(name="inp", bufs=1))
    expp = ctx.enter_context(tc.tile_pool(name="expp", bufs=3))
    tail1 = ctx.enter_context(tc.tile_pool(name="tail1", bufs=1))
    tail2 = ctx.enter_context(tc.tile_pool(name="tail2", bufs=2))
    usp = ctx.enter_context(tc.tile_pool(name="usp", bufs=3))
    mtp = ctx.enter_context(tc.tile_pool(name="mtp", bufs=2))
    qkps = ctx.enter_context(tc.tile_pool(name="qkps", bufs=2, space="PSUM"))
    avps = ctx.enter_context(tc.tile_pool(name="avps", bufs=2, space="PSUM"))
    mps = ctx.enter_context(tc.tile_pool(name="mps", bufs=1, space="PSUM"))
    dramp = ctx.enter_context(tc.tile_pool(name="dramp", bufs=2, space="DRAM"))

    # ---- inputs -----------------------------------------------------------
    # d-major layout [32, 8, 1024]: partition = head-dim d, free = (head, tok)
    fs32 = inp.tile([D, H, N], F32, tag="fs32")
    ft32 = inp.tile([D, H, N], F32, tag="ft32")
    for h in range(H):
        nc.sync.dma_start(
            out=fs32[:, h, :],
            in_=fs[32 * h:32 * (h + 1), :].rearrange("a b -> a b"),
        )
        nc.sync.dma_start(
            out=ft32[:, h, :],
            in_=ft[32 * h:32 * (h + 1), :].rearrange("a b -> a b"),
        )
    # natural-layout f_t for the loss tail: rows (h d) packed 4 heads/group
    ftt = []
    for g in range(2):
        t = inp.tile([128, N], F32, tag=f"ftt{g}")
        nc.sync.dma_start(out=t, in_=ft[128 * g:128 * (g + 1), :])
        ftt.append(t)

    # bf16 casts into augmented per-head tiles, REPLICATED at partition
    # bases 0 and 64 so K<=33 matmuls can run 2-way row-packed:
    #   rows 0..31 / 64..95  = qT (fsa) or kT (fta)
    #   row 32 / 96          = -rowmax (fsa, written per head) or 1.0 (fta)
    fsa = []
    fta = []
    for h in range(H):
        a = inp.tile([64 + D + 1, N], BF16, tag=f"fsa{h}")
        b = inp.tile([64 + D + 1, N], BF16, tag=f"fta{h}")
        nc.scalar.copy(a[0:D, :], fs32[:, h, :])
        nc.gpsimd.tensor_copy(b[0:D, :], ft32[:, h, :])
        # replicate to base 64 (partition shift => DMA)
        nc.sync.dma_start(out=a[64:64 + D, :], in_=a[0:D, :])
        nc.sync.dma_start(out=b[64:64 + D, :], in_=b[0:D, :])
        nc.gpsimd.memset(b[D:D + 1, :], 1.0)
        nc.gpsimd.memset(b[64 + D:64 + D + 1, :], 1.0)
        fsa.append(a)
        fta.append(b)

    # v tokens [j, d]: one batched xbar transpose per head into a dense
    # staging tile, then one strided DMA interleaves the ones column.
    vtok = inp.tile([128, H * 8, D + 1], BF16, tag="vtok")
    nc.gpsimd.memset(vtok[:, :, D:D + 1], 1.0)
    for h in range(H):
        vst = usp.tile([128, 8, D], BF16, tag="vst")
        nc.sync.dma_start_transpose(out=vst, in_=fsa[h][0:D, :])
        nc.sync.dma_start(
            out=vtok[:, h * 8:h * 8 + 8, 0:D], in_=vst
        )

    # ---- pipelined per-head emission --------------------------------------
    def emit_mpass(h):
        # exact row maxes in [i, j] orientation, 2-way row-packed:
        # groups {0,1} do (it, jh=0), groups {2,3} do (it, jh=1).
        # m_bf[p, it] = -max_j sim[i = 128*it + p, j]   (bf16)
        m_bf = mtp.tile([128, 32], BF16, tag="mbf")
        nc.gpsimd.memset(m_bf[:, 8:32], 0.0)
        for it in range(8):
            m_ps = mps.tile([128, 2, 512], F32, tag="mps")
            nc.tensor.matmul(
                m_ps[:, 0, :],
                lhsT=fsa[h][0:D, 128 * it:128 * (it + 1)],
                rhs=fta[h][0:D, 0:512],
                start=True,
                stop=True,
                tile_position=(0, 0),
            )
            nc.tensor.matmul(
                m_ps[:, 1, :],
                lhsT=fsa[h][64:64 + D, 128 * it:128 * (it + 1)],
                rhs=fta[h][64:64 + D, 512:1024],
                start=True,
                stop=True,
                tile_position=(64, 0),
            )
            nc.vector.tensor_reduce(
                out=m_bf[:, it:it + 1],
                in_=m_ps,
                axis=mybir.AxisListType.XY,
                op=ALU.max,
                negate=True,
            )
        # 32x32 block transpose: m_tr[32a + it, c] = m_bf[32a + c, it]
        m_tr = mtp.tile([128, 32], BF16, tag="mtr")
        nc.vector.transpose(m_tr, m_bf)
        # scatter -m into fsa rows 32 and 96: i = 128*it + 32a + c
        for base in (D, 64 + D):
            row = fsa[h][base:base + 1, :].rearrange(
                "q (it a c) -> q it a c", it=8, a=4
            )
            for a in range(4):
                nc.sync.dma_start(
                    out=row[:, :, a, :],
                    in_=m_tr[32 * a:32 * a + 8, :],
                )

    def emit_main(h, u_pack, s_pack):
        hh = h % 4
        # main pass: simT = [kT; 1]^T [qT; -m] (2-way row-packed over j
        # pairs), exp, then [v|1] matmul (2-way col-packed over i-halves
        # into one psum bank: ih0 -> rows 0..32, ih1 -> rows 64..96).
        av = avps.tile([128, 512], F32, tag="av")
        for jp in range(4):
            exs = []
            for ih in range(2):
                qk = qkps.tile([128, 2, 512], F32, tag="qk", name=f"qk{h}_{jp}_{ih}")
                nc.tensor.matmul(
                    qk[:, 0, :],
                    lhsT=fta[h][0:D + 1, 256 * jp:256 * jp + 128],
                    rhs=fsa[h][0:D + 1, 512 * ih:512 * (ih + 1)],
                    start=True,
                    stop=True,
                    tile_position=(0, 0),
                )
                nc.tensor.matmul(
                    qk[:, 1, :],
                    lhsT=fta[h][64:64 + D + 1, 256 * jp + 128:256 * jp + 256],
                    rhs=fsa[h][64:64 + D + 1, 512 * ih:512 * (ih + 1)],
                    start=True,
                    stop=True,
                    tile_position=(64, 0),
                )
                ex = expp.tile([128, 2, 512], BF16, tag="ex", name=f"ex{h}_{jp}_{ih}")
                nc.scalar.activation(out=ex, in_=qk, func=AF.Exp)
                exs.append(ex)
            # AV matmuls: adjacent col-packed pairs (ih0, ih1) per j
            for jj in range(2):
                for ih in range(2):
                    nc.tensor.matmul(
                        av[64 * ih:64 * ih + D + 1, :],
                        lhsT=vtok[:, h * 8 + 2 * jp + jj, :],
                        rhs=exs[ih][:, jj, :],
                        start=(jp == 0 and jj == 0),
                        stop=(jp == 3 and jj == 1),
                        tile_position=(0, 64 * ih),
                    )
        # drain this head's [u; s]: ACT psum->sbuf, then DMA packs the
        # 4 heads of the group into 128-partition tail buffers
        u_s = usp.tile([128, 512], F32, tag="us")
        for ih in range(2):
            nc.scalar.copy(
                u_s[64 * ih:64 * ih + D + 1, :], av[64 * ih:64 * ih + D + 1, :]
            )
        for ih in range(2):
            nc.scalar.dma_start(
                out=u_pack[32 * hh:32 * (hh + 1), 512 * ih:512 * (ih + 1)],
                in_=u_s[64 * ih:64 * ih + D, :],
            )
            nc.scalar.dma_start(
                out=s_pack[hh:hh + 1, 512 * ih:512 * (ih + 1)],
                in_=u_s[64 * ih + D:64 * ih + D + 1, :],
            )

    # ---- main loop (software-pipelined: m-pass runs one head ahead) -------
    acc_prev = None
    packs = {}
    for g in range(2):
        packs[g] = (
            tail2.tile([128, N], F32, tag="upack", name=f"upack{g}"),
            tail2.tile([4, N], F32, tag="spack", name=f"spack{g}"),
        )
    emit_mpass(0)
    for h in range(H):
        if h + 1 < H:
            emit_mpass(h + 1)
        emit_main(h, *packs[h // 4])
        if h % 4 != 3:
            continue
        g = h // 4
        u_pack, s_pack = packs[g]

        # ---- loss tail for this 4-head group ------------------------------
        r_pack = tail1.tile([4, N], F32, tag="rpack")
        nc.vector.reciprocal_approx_fast(out=r_pack, in_=s_pack)
        # broadcast r rows across partitions via a DRAM bounce
        r_dram = dramp.tile([4, N], F32, tag="rdram")
        nc.sync.dma_start(out=r_dram, in_=r_pack)
        r_b = tail2.tile([128, N], F32, tag="rb")
        for m in range(4):
            nc.sync.dma_start(
                out=r_b[32 * m:32 * (m + 1), :],
                in_=r_dram[m:m + 1, :].to_broadcast([32, N]),
            )
        o = tail1.tile([128, N], F32, tag="o")
        nc.vector.tensor_mul(o, u_pack, r_b)
        e = tail1.tile([128, N], F32, tag="e")
        nc.vector.tensor_sub(e, o, ftt[g])
        esq = tail1.tile([128, N], F32, tag="esq")
        nc.vector.tensor_mul(esq, e, e)
        acc = tail1.tile([128, 1], F32, tag=f"acc{g}")
        nc.vector.tensor_reduce(
            out=acc, in_=esq, axis=mybir.AxisListType.X, op=ALU.add
        )
        if acc_prev is not None:
            acc2 = tail1.tile([128, 1], F32, tag="accsum")
            nc.vector.tensor_add(acc2, acc, acc_prev)
            acc = acc2
        acc_prev = acc

    nc.sync.dma_start(out=out_dram, in_=acc_prev)


def build():
    nc = bacc.Bacc(
        "TRN2",
        target_bir_lowering=False,
        debug=False,
        num_devices=NCORES,
    )
    fs = nc.dram_tensor("fs", [C, N], F32, kind="ExternalInput")
    ft = nc.dram_tensor("ft", [C, N], F32, kind="ExternalInput")
    out = nc.dram_tensor("out", [128, 1], F32, kind="ExternalOutput")
    from contextlib import ExitStack

    with tile.TileContext(nc) as tc:
        with ExitStack() as ctx:
            _body(ctx, tc, fs.ap(), ft.ap(), out.ap())
    nc.compile()
    return nc


_CACHE = {}


def _get_nc():
    if "nc" not in _CACHE:
        _CACHE["nc"] = build()
    return _CACHE["nc"]


def run(f_s, f_t, trace=False):
    """Run on 8 NeuronCores; returns (loss_scalar, BassKernelResults)."""
    f_s = np.ascontiguousarray(np.asarray(f_s, dtype=np.float32))
    f_t = np.ascontiguousarray(np.asarray(f_t, dtype=np.float32))
    assert f_s.shape == (B, C, 32, 32) and f_t.shape == (B, C, 32, 32)
    nc = _get_nc()
    in_maps = [
        {
            "fs": f_s[b].reshape(C, N),
            "ft": f_t[b].reshape(C, N),
        }
        for b in range(B)
    ]
    res = run_bass_kernel_spmd(
        nc, in_maps, core_ids=list(range(NCORES)), trace=trace
    )
    total = np.float64(0.0)
    for r in res.results:
        total += np.asarray(r["out"], dtype=np.float64).sum()
    loss = np.float32(total / TOTAL_ELEMS)
    return loss, res


def kernel(f_s, f_t):
    loss, _ = run(f_s, f_t, trace=False)
    return loss


# revision 42
# speedup vs baseline: 1.1740x; 1.1740x over previous
"""Trainium2 Bass kernel for nn_Distiller attention-distillation loss.

Computes, for f_s, f_t of shape [8, 256, 32, 32]:
    q = k_tokens(f_s), k = tokens(f_t), v = tokens(f_s)   (8 heads, d=32, n=1024)
    out = softmax(q @ k^T) @ v          (per batch, per head; unscaled logits)
    loss = mean((out_img - f_t)^2)      (scalar)

Sharding: data-parallel over batch b — one batch element per NeuronCore (8
cores).  Each core computes its partial sum of squared errors; the host sums
the 8 partials and divides by the element count.  The mean is layout
invariant, so the loss is computed in token space and the final
'b h (x y) d -> b (h d) x y' rearrange is never materialized.

Per-core algorithm (all in [d, n]-major "transposed token" layouts so that
no input transposes are needed):
  simT[j, i] = sum_d kT[d, j] * qT[d, i]        (PE, bf16 inputs, fp32 psum)
  expT = exp(simT)                               (ACT, psum -> sbuf bf16)
  [u; s][d_aug, i] = [v_tok | 1s]^T-style matmul: stationary = v_tok[j, 33]
       (v tokens with an appended ones-column), moving = expT chunks,
       accumulated over j in psum.  Row 32 is the softmax denominator s[i].
  loss_part += sum((u/s - tT)^2)                 (DVE + custom ops)
"""

import numpy as np

import concourse.bass as bass
import concourse.bacc as bacc
import concourse.tile as tile
import concourse.mybir as mybir
from concourse.bass_utils import run_bass_kernel_spmd

F32 = mybir.dt.float32
BF16 = mybir.dt.bfloat16
AF = mybir.ActivationFunctionType
ALU = mybir.AluOpType

B = 8          # batch (== number of cores)
H = 8          # heads
D = 32         # head dim
N = 1024       # tokens (32*32)
C = H * D      # channels = 256
NCORES = 8
TOTAL_ELEMS = B * C * 32 * 32  # 2097152


def _body(ctx, tc, fs, ft, seld, out_dram):
    nc = tc.nc

    inp = ctx.enter_context(tc.tile_pool(name="inp", bufs=1))
    expp = ctx.enter_context(tc.tile_pool(name="expp", bufs=3))
    tail1 = ctx.enter_context(tc.tile_pool(name="tail1", bufs=1))
    tail2 = ctx.enter_context(tc.tile_pool(name="tail2", bufs=2))
    usp = ctx.enter_context(tc.tile_pool(name="usp", bufs=3))
    mtp = ctx.enter_context(tc.tile_pool(name="mtp", bufs=3))
    qkps = ctx.enter_context(tc.tile_pool(name="qkps", bufs=3, space="PSUM"))
    avps = ctx.enter_context(tc.tile_pool(name="avps", bufs=2, space="PSUM"))
    mps = qkps

    # ---- inputs -----------------------------------------------------------
    # d-major layout [32, 8, 1024]: partition = head-dim d, free = (head, tok)
    fs32 = inp.tile([D, H, N], F32, tag="fs32")
    ft32 = inp.tile([D, H, N], F32, tag="ft32")
    for h in range(H):
        nc.sync.dma_start(
            out=fs32[:, h, :],
            in_=fs[32 * h:32 * (h + 1), :].rearrange("a b -> a b"),
        )
        nc.sync.dma_start(
            out=ft32[:, h, :],
            in_=ft[32 * h:32 * (h + 1), :].rearrange("a b -> a b"),
        )
    # natural-layout f_t for the loss tail: rows (h d) packed 4 heads/group
    ftt = []
    for g in range(2):
        t = inp.tile([128, N], F32, tag=f"ftt{g}")
        nc.sync.dma_start(out=t, in_=ft[128 * g:128 * (g + 1), :])
        ftt.append(t)

    # bf16 casts into augmented per-head tiles, REPLICATED at partition
    # bases 0 and 64 so K<=33 matmuls can run 2-way row-packed:
    #   rows 0..31 / 64..95  = qT (fsa) or kT (fta)
    #   row 32 / 96          = -rowmax (fsa, written per head) or 1.0 (fta)
    fsa = []
    fta = []
    for h in range(H):
        a = inp.tile([64 + D + 1, N], BF16, tag=f"fsa{h}")
        b = inp.tile([64 + D + 1, N], BF16, tag=f"fta{h}")
        nc.scalar.copy(a[0:D, :], fs32[:, h, :])
        nc.scalar.copy(b[0:D, :], ft32[:, h, :])
        # replicate to base 64 (partition shift => DMA)
        nc.sync.dma_start(out=a[64:64 + D, :], in_=a[0:D, :])
        nc.sync.dma_start(out=b[64:64 + D, :], in_=b[0:D, :])
        nc.gpsimd.memset(b[D:D + 1, :], 1.0)
        nc.gpsimd.memset(b[64 + D:64 + D + 1, :], 1.0)
        fsa.append(a)
        fta.append(b)

    # v tokens [j, d]: one batched xbar transpose per head into a dense
    # staging tile, then one strided DMA interleaves the ones column.
    # selector for the tail broadcast matmul: sel[k, 32k + c] = 1
    sel = inp.tile([4, 128], BF16, tag="sel")
    nc.sync.dma_start(out=sel, in_=seld)

    vtok = inp.tile([128, H * 8, D + 1], BF16, tag="vtok")
    nc.gpsimd.memset(vtok[:, :, D:D + 1], 1.0)
    for h in range(H):
        vst = usp.tile([128, 8, D], BF16, tag="vst")
        nc.sync.dma_start_transpose(out=vst, in_=fsa[h][0:D, :])
        nc.sync.dma_start(
            out=vtok[:, h * 8:h * 8 + 8, 0:D], in_=vst
        )

    # ---- pipelined per-head emission --------------------------------------
    def emit_mpass(h):
        # exact row maxes in [i, j] orientation, 2-way row-packed:
        # groups {0,1} do (it, jh=0), groups {2,3} do (it, jh=1).
        # m_bf[p, it] = -max_j sim[i = 128*it + p, j]   (bf16)
        m_bf = mtp.tile([128, 32], BF16, tag="mbf")
        nc.gpsimd.memset(m_bf[:, 8:32], 0.0)
        for it in range(8):
            m_ps = mps.tile([128, 2, 512], F32, tag="qk", name=f"mps{h}_{it}")
            nc.tensor.matmul(
                m_ps[:, 0, :],
                lhsT=fsa[h][0:D, 128 * it:128 * (it + 1)],
                rhs=fta[h][0:D, 0:512],
                start=True,
                stop=True,
                tile_position=(0, 0),
            )
            nc.tensor.matmul(
                m_ps[:, 1, :],
                lhsT=fsa[h][64:64 + D, 128 * it:128 * (it + 1)],
                rhs=fta[h][64:64 + D, 512:1024],
                start=True,
                stop=True,
                tile_position=(64, 0),
            )
            nc.vector.tensor_reduce(
                out=m_bf[:, it:it + 1],
                in_=m_ps,
                axis=mybir.AxisListType.XY,
                op=ALU.max,
                negate=True,
            )
        # 32x32 block transpose: m_tr[32a + it, c] = m_bf[32a + c, it]
        m_tr = mtp.tile([128, 32], BF16, tag="mtr")
        nc.vector.transpose(m_tr, m_bf)
        # scatter -m into fsa rows 32 and 96: i = 128*it + 32a + c
        for base in (D, 64 + D):
            row = fsa[h][base:base + 1, :].rearrange(
                "q (it a c) -> q it a c", it=8, a=4
            )
            for a in range(4):
                nc.sync.dma_start(
                    out=row[:, :, a, :],
                    in_=m_tr[32 * a:32 * a + 8, :],
                )

    def emit_main(h, u_pack, s_pack):
        hh = h % 4
        # main pass: simT = [kT; 1]^T [qT; -m] (2-way row-packed over j
        # pairs), exp, then [v|1] matmul (2-way col-packed over i-halves
        # into one psum bank: ih0 -> rows 0..32, ih1 -> rows 64..96).
        av = avps.tile([128, 512], F32, tag="av")
        for jp in range(4):
            exs = []
            for ih in range(2):
                qk = qkps.tile([128, 2, 512], F32, tag="qk", name=f"qk{h}_{jp}_{ih}")
                nc.tensor.matmul(
                    qk[:, 0, :],
                    lhsT=fta[h][0:D + 1, 256 * jp:256 * jp + 128],
                    rhs=fsa[h][0:D + 1, 512 * ih:512 * (ih + 1)],
                    start=True,
                    stop=True,
                    tile_position=(0, 0),
                )
                nc.tensor.matmul(
                    qk[:, 1, :],
                    lhsT=fta[h][64:64 + D + 1, 256 * jp + 128:256 * jp + 256],
                    rhs=fsa[h][64:64 + D + 1, 512 * ih:512 * (ih + 1)],
                    start=True,
                    stop=True,
                    tile_position=(64, 0),
                )
                ex = expp.tile([128, 2, 512], BF16, tag="ex", name=f"ex{h}_{jp}_{ih}")
                nc.scalar.activation(out=ex, in_=qk, func=AF.Exp)
                exs.append(ex)
            # AV matmuls: adjacent col-packed pairs (ih0, ih1) per j
            for jj in range(2):
                for ih in range(2):
                    nc.tensor.matmul(
                        av[64 * ih:64 * ih + D + 1, :],
                        lhsT=vtok[:, h * 8 + 2 * jp + jj, :],
                        rhs=exs[ih][:, jj, :],
                        start=(jp == 0 and jj == 0),
                        stop=(jp == 3 and jj == 1),
                        tile_position=(0, 64 * ih),
                        skip_group_check=True,
                    )
        # drain this head's [u; s]: ACT psum->sbuf, then DMA packs the
        # 4 heads of the group into 128-partition tail buffers
        u_s = usp.tile([128, 512], F32, tag="us")
        for ih in range(2):
            nc.scalar.copy(
                u_s[64 * ih:64 * ih + D + 1, :], av[64 * ih:64 * ih + D + 1, :]
            )
        for ih in range(2):
            nc.scalar.dma_start(
                out=u_pack[32 * hh:32 * (hh + 1), 512 * ih:512 * (ih + 1)],
                in_=u_s[64 * ih:64 * ih + D, :],
            )
            nc.scalar.dma_start(
                out=s_pack[hh:hh + 1, 512 * ih:512 * (ih + 1)],
                in_=u_s[64 * ih + D:64 * ih + D + 1, :],
            )

    # ---- main loop (software-pipelined: m-pass runs one head ahead) -------
    acc_prev = None
    packs = {}
    for g in range(2):
        packs[g] = (
            tail2.tile([128, N], F32, tag="upack", name=f"upack{g}"),
            tail2.tile([4, N], F32, tag="spack", name=f"spack{g}"),
        )
    emit_mpass(0)
    emit_mpass(1)
    for h in range(H):
        if h + 2 < H:
            emit_mpass(h + 2)
        emit_main(h, *packs[h // 4])
        if h % 4 != 3:
            continue
        g = h // 4
        u_pack, s_pack = packs[g]

        # ---- loss tail for this 4-head group ------------------------------
        r_pack = tail1.tile([4, N], F32, tag="rpack")
        nc.vector.reciprocal_approx_fast(out=r_pack, in_=s_pack)
        # broadcast r rows across partition groups via a tiny PE matmul:
        # r_b[32k + c, i] = r_pack[k, i]
        r_bf = tail1.tile([4, N], BF16, tag="rbf")
        nc.vector.tensor_copy(r_bf, r_pack)
        r_b = qkps.tile([128, 2, 512], F32, tag="qk", name=f"rb{g}")
        for ih in range(2):
            nc.tensor.matmul(
                r_b[:, ih, :],
                lhsT=sel,
                rhs=r_bf[:, 512 * ih:512 * (ih + 1)],
                start=True,
                stop=True,
            )
        o = tail1.tile([128, N], F32, tag="o")
        nc.vector.tensor_mul(o, u_pack, r_b.rearrange("p a b -> p (a b)"))
        e = tail1.tile([128, N], F32, tag="e")
        nc.vector.tensor_sub(e, o, ftt[g])
        esq = tail1.tile([128, N], F32, tag="esq")
        nc.vector.tensor_mul(esq, e, e)
        acc = tail1.tile([128, 1], F32, tag=f"acc{g}")
        nc.vector.tensor_reduce(
            out=acc, in_=esq, axis=mybir.AxisListType.X, op=ALU.add
        )
        if acc_prev is not None:
            acc2 = tail1.tile([128, 1], F32, tag="accsum")
            nc.vector.tensor_add(acc2, acc, acc_prev)
            acc = acc2
        acc_prev = acc

    nc.sync.dma_start(out=out_dram, in_=acc_prev)


def build():
    nc = bacc.Bacc(
        "TRN2",
        target_bir_lowering=False,
        debug=False,
        num_devices=NCORES,
    )
    fs = nc.dram_tensor("fs", [C, N], F32, kind="ExternalInput")
    ft = nc.dram_tensor("ft", [C, N], F32, kind="ExternalInput")
    seld = nc.dram_tensor("sel", [4, 128], BF16, kind="ExternalInput")
    out = nc.dram_tensor("out", [128, 1], F32, kind="ExternalOutput")
    from contextlib import ExitStack

    with tile.TileContext(nc) as tc:
        with ExitStack() as ctx:
            _body(ctx, tc, fs.ap(), ft.ap(), seld.ap(), out.ap())
    nc.compile()
    return nc


def _sel_np():
    import ml_dtypes

    s = np.zeros((4, 128), dtype=ml_dtypes.bfloat16)
    for k in range(4):
        s[k, 32 * k:32 * (k + 1)] = 1.0
    return s


_CACHE = {}


def _get_nc():
    if "nc" not in _CACHE:
        _CACHE["nc"] = build()
    return _CACHE["nc"]


def run(f_s, f_t, trace=False):
    """Run on 8 NeuronCores; returns (loss_scalar, BassKernelResults)."""
    f_s = np.ascontiguousarray(np.asarray(f_s, dtype=np.float32))
    f_t = np.ascontiguousarray(np.asarray(f_t, dtype=np.float32))
    assert f_s.shape == (B, C, 32, 32) and f_t.shape == (B, C, 32, 32)
    nc = _get_nc()
    sel = _sel_np()
    in_maps = [
        {
            "fs": f_s[b].reshape(C, N),
            "ft": f_t[b].reshape(C, N),
            "sel": sel,
        }
        for b in range(B)
    ]
    res = run_bass_kernel_spmd(
        nc, in_maps, core_ids=list(range(NCORES)), trace=trace
    )
    total = np.float64(0.0)
    for r in res.results:
        total += np.asarray(r["out"], dtype=np.float64).sum()
    loss = np.float32(total / TOTAL_ELEMS)
    return loss, res


def kernel(f_s, f_t):
    loss, _ = run(f_s, f_t, trace=False)
    return loss
